# revision 124
# baseline (speedup 1.0000x reference)
"""Trainium2 Bass kernel for nn_Attention_39015482916872.

Multi-head attention (B=2, N=2048, C=1024, H=16, D=64) with RoPE,
tensor-parallel over (batch, heads) across 8 NeuronCores: core c handles
batch c//4 and heads 4*(c%4)..4*(c%4)+3. Each core computes its heads'
QKV projection, RoPE, attention, and a partial output projection; the
host sums the 4 partials per batch (Megatron-style column-parallel
w_proj) and adds b_proj.

Device-side design notes:
 - All matmuls in bf16 (f32 accumulation in PSUM).
 - x is cast f32->bf16 by a DMA (SWDGE cast) into internal DRAM, then
   DMA-transposed (XBAR) into SBUF so C lands on partitions.
 - RoPE pairing (even,odd) is conjugated by a fixed permutation into
   [real32 | imag32] halves per head (folded into w_qkv rows host-side;
   cancels in q.k since Q and K share it) so the DVE ops stay
   32/64-partition aligned.
 - Scores are computed transposed (n_k on partitions); softmax uses no
   max-subtraction (scores ~ N(0,1), exp cannot overflow) and the
   denominator comes from a 65th all-ones column appended to V; the
   division is applied to the small (D x n_q) PV output.
"""

import sys
from contextlib import ExitStack, nullcontext

import numpy as np

if "/opt/trn_rl_repo" not in sys.path:
    sys.path.insert(0, "/opt/trn_rl_repo")
try:
    import concourse.bass as bass
except ImportError:
    sys.path.insert(0, "/root/.axon_site/_ro/trn_rl_repo")
    import concourse.bass as bass
import concourse.tile as tile
from concourse import bacc, mybir
from concourse.bass_utils import run_bass_kernel_spmd

F32 = mybir.dt.float32
BF16 = mybir.dt.bfloat16
F8 = mybir.dt.float8e4
AF = mybir.ActivationFunctionType
DR = mybir.MatmulPerfMode.DoubleRow

B, N, C, H, D = 2, 2048, 1024, 16, 64
N_CORES = 8
CORES_PER_BATCH = N_CORES // B          # 4
HPC = H // CORES_PER_BATCH              # 4 heads per core
WS = 32.0  # weight pre-scale: w_qkv ~ N(0, 1/32) -> ~N(0,1) avoids fp8 subnormals


def build_attn_kernel(nc, tc, ctx, N=2048, C=1024, HPC=4, D=64, NQ_BLK=512, scale=None, phases=3, drive_mode='E', SB=2):
    P = 128
    KC = C // P
    QK_CHUNKS = 2 * HPC * D // P
    VF = HPC * D
    NB = N // NQ_BLK
    NKC = N // P
    NPC = N // P
    if scale is None:
        # q and k each carry a WS factor from the pre-scaled weights
        scale = D ** -0.5 / (WS * WS)

    # QKV projection runs in fp8 (e4m3) with residual correction:
    # x = x1 + x2, w = w1 + w2 (both fp8 splits computed host-side), and the
    # psum accumulates x1w1 + x1w2 + x2w1 via DoubleRow matmuls (2 K-subtiles
    # per instruction at 0.5 cycles/row = 2x bf16 throughput). The dropped
    # x2w2 term is O(eps^2) ~ 0.1%.
    x1d = nc.dram_tensor("x1", [C, N], F8, kind="ExternalInput").ap()
    x2d = nc.dram_tensor("x2", [C, N], F8, kind="ExternalInput").ap()
    wqk1d = nc.dram_tensor("wqk1", [C, 2 * HPC * D], F8, kind="ExternalInput").ap()
    wqk2d = nc.dram_tensor("wqk2", [C, 2 * HPC * D], F8, kind="ExternalInput").ap()
    wv1d = nc.dram_tensor("wv1", [C, VF], F8, kind="ExternalInput").ap()
    wv2d = nc.dram_tensor("wv2", [C, VF], F8, kind="ExternalInput").ap()
    wpT = nc.dram_tensor("wpT", [VF, C], BF16, kind="ExternalInput").ap()
    cosT = nc.dram_tensor("cosT", [D // 2, N], BF16, kind="ExternalInput").ap()
    sinT = nc.dram_tensor("sinT", [D, N], BF16, kind="ExternalInput").ap()
    y = nc.dram_tensor("y", [N, C], BF16, kind="ExternalOutput").ap()

    persist = ctx.enter_context(tc.tile_pool(name="persist", bufs=1))
    psum_mm = ctx.enter_context(tc.tile_pool(name="psum_mm", bufs=2, space="PSUM"))
    # psum_s / psum_o are created lazily (after the boot pool closes) so the
    # opening QKV phase can use their banks for deeper psum pipelining
    psum_s = psum_o = None

    def make_attn_psum_pools():
        nonlocal psum_s, psum_o
        if psum_s is None:
            psum_s = ctx.enter_context(tc.tile_pool(name="psum_s", bufs=2, space="PSUM"))
            psum_o = ctx.enter_context(tc.tile_pool(name="psum_o", bufs=1, space="PSUM"))
    rope_tmp = ctx.enter_context(tc.tile_pool(name="rope_tmp", bufs=6))
    exp_pool = ctx.enter_context(tc.tile_pool(name="exp_pool", bufs=34))
    norm_pool = ctx.enter_context(tc.tile_pool(name="norm_pool", bufs=6))
    y_pool = ctx.enter_context(tc.tile_pool(name="y_pool", bufs=8))

    NH = max(1, N // 1024)   # n-halves of 1024
    HW_ = N // NH            # half width
    KH = KC // 2             # DoubleRow k-subtile pairs
    x1sb = [persist.tile([P, KC, HW_], F8, name=f"x1_{h}", tag=f"x1_{h}") for h in range(NH)]
    x2sb = [persist.tile([P, KC, HW_], F8, name=f"x2_{h}", tag=f"x2_{h}") for h in range(NH)]

    def x_slice(xsb, k2, n0, w):
        h = n0 // HW_
        assert (n0 + w - 1) // HW_ == h
        return xsb[h][:, 2 * k2:2 * k2 + 2, n0 - h * HW_:n0 - h * HW_ + w]
    wqk1_sb = persist.tile([P, KC, 2 * HPC * D], F8, tag="wqk1")
    wqk2_sb = persist.tile([P, KC, 2 * HPC * D], F8, tag="wqk2")
    wv1_sb = persist.tile([P, KC, VF], F8, tag="wv1")
    wv2_sb = persist.tile([P, KC, VF], F8, tag="wv2")
    wpT_sb = persist.tile([P, VF // P, C], BF16, tag="wp")
    cos_sb = persist.tile([P, N], BF16, tag="cos")
    sin_sb = persist.tile([P, N], BF16, tag="sin")
    qt = [[persist.tile([P, NQ_BLK], BF16, name=f"qt{i}_{j}", tag=f"qt{i}_{j}") for j in range(NB)] for i in range(QK_CHUNKS // 2)]
    kt = [[persist.tile([P, NQ_BLK], BF16, name=f"kt{i}_{j}", tag=f"kt{i}_{j}") for j in range(NB)] for i in range(QK_CHUNKS // 2)]
    # V augmented with 64 ones-columns: MM2 then yields [v-out | denominator
    # replicated across 64 partitions], so softmax normalize needs no
    # partition-broadcast (matmul cost is per moving row -- the extra PE
    # columns are free)
    vaug = [persist.tile([P, HPC, 2 * D], BF16, name=f"va{j}", tag=f"va{j}") for j in range(NPC)]
    # anorm split per (feature-chunk, nq-block) so phase3 readers only wait on
    # the normalize writes of their own nq block (Tile deps are tile-granular)
    anorm = [[persist.tile([P, NQ_BLK], BF16, name=f"an{i}_{j}", tag=f"an{i}_{j}")
              for j in range(NB)] for i in range(VF // P)]

    def anorm_slice(i, n0, w):
        j = n0 // NQ_BLK
        assert (n0 + w - 1) // NQ_BLK == j
        return anorm[i][j][:, n0 - j * NQ_BLK:n0 - j * NQ_BLK + w]

    # preload the exp activation table during the DMA prefix so the first
    # real softmax exp doesn't pay the ~2.7us ACT_TABLE_LOAD
    warm = persist.tile([1, 8], F32, tag="actwarm")
    nc.vector.memset(warm[:], 0.0)
    nc.scalar.activation(warm[:], warm[:], AF.Exp, scale=1.0)

    # DMA order tuned so the first qk_block's term-1 dependencies (wqk1 +
    # x1[half0]) land first, then term-2/3 operands, then cos/sin (needed by
    # the first rope), then half-1 x with wv, then wp (only needed in phase 3).
    XB = 2  # x chunks per DMA (batches HWDGE/SP issue overhead)

    def _xdma(xd, xsb, h, k, w=None):
        n0 = h * HW_
        w = w or XB
        nc.sync.dma_start(
            xsb[h][:, k:k + w, :],
            xd[k * P:(k + w) * P, n0:n0 + HW_].rearrange("(kc p) n -> p kc n", p=P),
        )

    HK = KC // 2
    nc.sync.dma_start(wqk1_sb[:, 0:HK, :], wqk1d[0:HK * P, :].rearrange("(kc p) f -> p kc f", p=P))
    for k in range(0, HK, XB):
        _xdma(x1d, x1sb, 0, k)
    nc.sync.dma_start(wqk1_sb[:, HK:KC, :], wqk1d[HK * P:, :].rearrange("(kc p) f -> p kc f", p=P))
    for k in range(HK, KC, XB):
        _xdma(x1d, x1sb, 0, k)
    nc.sync.dma_start(wqk2_sb[:], wqk2d.rearrange("(kc p) f -> p kc f", p=P))
    for k in range(0, KC, XB):
        _xdma(x2d, x2sb, 0, k)
    for g in range(P // (D // 2)):
        nc.sync.dma_start(cos_sb[g * (D // 2):(g + 1) * (D // 2), :], cosT[:, :])
    for g in range(P // D):
        nc.sync.dma_start(sin_sb[g * D:(g + 1) * D, :], sinT[:, :])
    nc.sync.dma_start(wv1_sb[:], wv1d.rearrange("(kc p) f -> p kc f", p=P))
    nc.sync.dma_start(wv2_sb[:], wv2d.rearrange("(kc p) f -> p kc f", p=P))
    if NH > 1:
        for k in range(0, KC, 4):
            _xdma(x1d, x1sb, 1, k, w=4)
            _xdma(x2d, x2sb, 1, k, w=4)
    nc.sync.dma_start(wpT_sb[:], wpT.rearrange("(vc p) f -> p vc f", p=P))
    HALF0_FIRST = True

    QKV_TERMS = 3  # (x1,w1), (x1,w2), (x2,w1)

    def v_chunk(j, pool=None, evict_dve=False):
        pv = (pool or psum_mm).tile([P, VF], F32, name="pmm", tag="pmm", padded_shape=[P, 512])
        for t, (xsb, wsb) in enumerate(((x1sb, wv1_sb), (x1sb, wv2_sb), (x2sb, wv1_sb))):
            for k2 in range(KH):
                nc.tensor.matmul(
                    pv[:],
                    lhsT=x_slice(xsb, k2, j * P, P),
                    rhs=wsb[:, 2 * k2:2 * k2 + 2, :],
                    start=(t == 0 and k2 == 0),
                    stop=(t == QKV_TERMS - 1 and k2 == KH - 1),
                    perf_mode=DR,
                )
        nc.vector.memset(vaug[j][:, :, D:2 * D], 1.0)
        if evict_dve:
            nc.vector.tensor_copy(vaug[j][:, :, 0:D], pv[:].rearrange("p (h d) -> p h d", d=D))
        else:
            nc.any.tensor_copy(vaug[j][:, :, 0:D], pv[:].rearrange("p (h d) -> p h d", d=D))

    if phases < 0.4:
        return
    if phases < 0.8:
        for j in range(NPC):
            v_chunk(j)
        return
    # Phase 1b: Q,K feat-major + RoPE (chunk feat layout: [h0r h0i h1r h1i])
    def rope_chunk(psum_c, dst, j, raw_on_dve=False):
        nb = j * NQ_BLK
        cs = cos_sb[:, nb:nb + NQ_BLK]
        sn = sin_sb[:, nb:nb + NQ_BLK]
        raw = rope_tmp.tile([P, NQ_BLK], BF16, tag="raw")
        if raw_on_dve:
            # inside the Act-bound attention region every Act cycle counts
            nc.vector.tensor_copy(raw[:], psum_c[:])
        else:
            nc.any.tensor_copy(raw[:], psum_c[:])
        tA = rope_tmp.tile([P, NQ_BLK], BF16, tag="tA")
        tB = rope_tmp.tile([P, NQ_BLK], BF16, tag="tB")
        nc.vector.tensor_mul(tA[:], raw[:], cs)
        # swapped sin product: out rows swap r<->i; the +/- sign is folded into
        # the sin table (rows [g*64:g*64+32] = +sin, [g*64+32:g*64+64] = -sin),
        # so DVE 2-input base partitions always match (walrus NCC_IBIR297).
        for g in range(2):
            b0 = 64 * g
            nc.vector.tensor_mul(tB[b0:b0 + 32, :], raw[b0 + 32:b0 + 64, :], sn[b0 + 32:b0 + 64, :])
            nc.vector.tensor_mul(tB[b0 + 32:b0 + 64, :], raw[b0:b0 + 32, :], sn[b0:b0 + 32, :])
        nc.vector.tensor_add(dst[:], tA[:], tB[:])

    def qk_pieces(qk, i, j, n_pieces):
        """Split a qk_block's 12 DoubleRow matmuls (+rope) into n_pieces
        emission thunks, for interleaving into an attention block's kk loop
        (PE filler while the Act engine works through the exp backlog)."""
        dst_t = qt[i][j] if qk == 0 else kt[i][j]
        fbase = (qk * (QK_CHUNKS // 2) + i) * P
        pairs = [(t, k2)
                 for t in range(QKV_TERMS) for k2 in range(KH)]
        terms = ((x1sb, wqk1_sb), (x1sb, wqk2_sb), (x2sb, wqk1_sb))
        cell = {}
        chunk = -(-len(pairs) // n_pieces)

        def make(p):
            def emit():
                if "pq" not in cell:
                    cell["pq"] = psum_mm.tile([P, NQ_BLK], F32, name="pmm", tag="pmm", padded_shape=[P, 512])
                pqk = cell["pq"]
                for idx in range(p * chunk, min((p + 1) * chunk, len(pairs))):
                    t, k2 = pairs[idx]
                    xsb, wsb = terms[t]
                    nc.tensor.matmul(
                        pqk[:],
                        lhsT=wsb[:, 2 * k2:2 * k2 + 2, fbase:fbase + P],
                        rhs=x_slice(xsb, k2, j * NQ_BLK, NQ_BLK),
                        start=(idx == 0),
                        stop=(idx == len(pairs) - 1),
                        perf_mode=DR,
                    )
                if (p + 1) * chunk >= len(pairs):
                    rope_chunk(pqk, dst_t[:], j, raw_on_dve=True)
            return emit
        return [make(p) for p in range(n_pieces)]

    def qk_block(qk, i, j, pool=None):
        dst_t = qt[i][j] if qk == 0 else kt[i][j]
        fbase = (qk * (QK_CHUNKS // 2) + i) * P
        pqk = (pool or psum_mm).tile([P, NQ_BLK], F32, name="pmm", tag="pmm", padded_shape=[P, 512])
        for t, (xsb, wsb) in enumerate(((x1sb, wqk1_sb), (x1sb, wqk2_sb), (x2sb, wqk1_sb))):
            for k2 in range(KH):
                nc.tensor.matmul(
                    pqk[:],
                    lhsT=wsb[:, 2 * k2:2 * k2 + 2, fbase:fbase + P],
                    rhs=x_slice(xsb, k2, j * NQ_BLK, NQ_BLK),
                    start=(t == 0 and k2 == 0),
                    stop=(t == QKV_TERMS - 1 and k2 == KH - 1),
                    perf_mode=DR,
                )
        rope_chunk(pqk, dst_t[:], j)

    # Phase 2: attention per head, scores transposed (n_k on partitions).
    # psum_s batches SB nk-chunks so each exp covers SB*NQ_BLK elements
    # (amortizes the ~352-cycle ACT per-instruction overhead).
    def attention_mms(i, j, filler=(), filler_at=None, hi=False):
        # Both heads of chunk i at n_q block j. The two MM1s at each kk use
        # disjoint PE row strips (rows 0-63 vs 64-127), issued back-to-back so
        # the hardware runs them concurrently; one exp covers both heads.
        # `filler` thunks (next qk blocks' matmul pieces) are emitted spread
        # through the kk loop: the attention stretch is locally Act-bound, so
        # they fill PE slots while MM1 waits for the exp to free psum_s.
        h0, h1 = 2 * i, 2 * i + 1
        po0 = psum_o.tile([2 * D, NQ_BLK], F32, name="po0", tag="po0")
        po1 = psum_o.tile([2 * D, NQ_BLK], F32, name="po1", tag="po1")
        nf = len(filler)
        for kk in range(NKC):
            if filler_at:
                for th in filler_at.get(kk, ()):
                    th()
            with tc.high_priority() if hi else nullcontext():
                ps = psum_s.tile([P, 2, NQ_BLK], F32, tag="ps")
                kb, kc0 = divmod(kk * P, NQ_BLK)
                for g, h in ((0, h0), (1, h1)):
                    hb = 64 * g
                    nc.tensor.matmul(
                        ps[:, g, :],
                        lhsT=kt[i][kb][hb:hb + 64, kc0:kc0 + P],
                        rhs=qt[i][j][hb:hb + 64, :],
                        start=True,
                        stop=True,
                    )
                es = exp_pool.tile([P, 2, NQ_BLK], BF16, tag="es")
                nc.scalar.activation(es[:], ps[:], AF.Exp, scale=float(scale))
                for g, h, po in ((0, h0, po0), (1, h1, po1)):
                    nc.tensor.matmul(
                        po[:],
                        lhsT=vaug[kk][:, h, :],
                        rhs=es[:, g, :],
                        start=(kk == 0),
                        stop=(kk == NKC - 1),
                    )
            if nf:
                for p in range(nf):
                    if ((p + 1) * NKC) // nf - 1 == kk:
                        filler[p]()
        return po0, po1

    def attention_norm(i, j, po0, po1):
        # recip on DVE, multiply on gpsimd: anorm's writer engine is Pool, so
        # phase3's coarse engine-counter waits don't drag in the DVE rope queue
        h0, h1 = 2 * i, 2 * i + 1
        for h, po in ((h0, po0), (h1, po1)):
            bcast = norm_pool.tile([D, NQ_BLK], F32, tag="bcast")
            nc.vector.reciprocal(bcast[:], po[D:2 * D, :])
            dst = anorm[(h * D) // P][j]
            db = (h * D) % P
            nc.vector.tensor_mul(dst[db:db + D, :], po[0:D, :], bcast[:])

    def attention_pair_block(i, j):
        po0, po1 = attention_mms(i, j)
        attention_norm(i, j, po0, po1)


    # Phase 3: partial output projection (natural layout, n on partitions)
    OB = min(512, C)
    NOB = C // OB
    def phase3_rows(j, evict_dve=False):
        for ob in range(NOB):
            py = psum_mm.tile([P, OB], F32, name="pmm", tag="pmm", padded_shape=[P, 512])
            for i in range(VF // P):
                nc.tensor.matmul(
                    py[:],
                    lhsT=anorm_slice(i, j * P, P),
                    rhs=wpT_sb[:, i, ob * OB:(ob + 1) * OB],
                    start=(i == 0),
                    stop=(i == VF // P - 1),
                )
            yt = y_pool.tile([P, OB], BF16, tag="yt")
            if evict_dve:
                nc.vector.tensor_copy(yt[:], py[:])
            else:
                nc.any.tensor_copy(yt[:], py[:])
            nc.sync.dma_start(y[j * P:(j + 1) * P, ob * OB:(ob + 1) * OB], yt[:])

    NPB = NQ_BLK // P       # 128-row chunks per nq block
    NCH = QK_CHUNKS // 2
    LH = NCH - 1
    if drive_mode != "E":
        make_attn_psum_pools()
    if drive_mode in ("A", "D"):
        # V, then per chunk: K, Q, attention; phase3 interleaved (A) or last (D)
        for j in range(NPC):
            v_chunk(j)
        for i in range(NCH):
            for j in range(NB):
                qk_block(1, i, j)
            for j in range(NB):
                qk_block(0, i, j)
            if phases < 2:
                continue
            for j in range(NB):
                attention_pair_block(i, j)
                if phases >= 3 and i == LH and drive_mode == "A":
                    for jj in range(j * NPB, (j + 1) * NPB):
                        phase3_rows(jj)
        if phases >= 3 and drive_mode == "D":
            for jj in range(NPC):
                phase3_rows(jj)
    elif drive_mode == "B":
        # V, all QK chunks, then all attention
        for j in range(NPC):
            v_chunk(j)
        for i in range(NCH):
            for j in range(NB):
                qk_block(1, i, j)
            for j in range(NB):
                qk_block(0, i, j)
        if phases >= 2:
            for i in range(NCH):
                for j in range(NB):
                    attention_pair_block(i, j)
                    if phases >= 3 and i == LH:
                        for jj in range(j * NPB, (j + 1) * NPB):
                            phase3_rows(jj)
    elif drive_mode == "E":
        # earliest-exp: half-0 QKV in a deep scoped psum pool, then half-1
        # QK; second-half V chunks ride as fillers inside attention(0,0)
        # (only needed from kk=8). Remaining qk blocks are interleaved into
        # the attention kk loops as PE filler (the attention stretch is
        # Act-bound, so filler matmuls are free).
        pre = set()
        NBH = NB // 2
        # Only the first 4 (DMA-paced) qk blocks use the scoped boot pool:
        # psum_s/psum_o allocate in the released boot zone, so the boot pool
        # must close EARLY or its release dependency gates the first exp.
        # Later boot work (V chunks, half-1 K blocks) runs on psum_mm and
        # overlaps the attention stream.
        with tc.tile_pool(name="psum_boot", bufs=6, space="PSUM") as boot:
            for j in range(NBH):
                qk_block(1, 0, j, pool=boot)
            for j in range(NBH):
                qk_block(0, 0, j, pool=boot)
                pre.add(j)
        make_attn_psum_pools()
        for j in range(NPC // 2):
            v_chunk(j)
        for j in range(NBH, NB):
            qk_block(1, 0, j)
        v_rest = list(range(NPC // 2, NPC))
        if phases >= 2:
            emitted_q = {(0, j) for j in pre}
            for i in range(NCH):
                for j in range(NB):
                    if (i, j) not in emitted_q:
                        qk_block(0, i, j)
                    fill = []
                    if phases >= 3 and i == NCH - 1 and j >= 2:
                        def p3pair(jjs):
                            def emit():
                                for jj in jjs:
                                    phase3_rows(jj, evict_dve=True)
                            return emit
                        base = (j - 2) * NPB
                        fill += [p3pair([base, base + 1]), p3pair([base + 2, base + 3])]
                    if i == 0 and j == 0 and v_rest:
                        def vpair(js):
                            def emit():
                                for jj in js:
                                    v_chunk(jj)
                            return emit
                        fill += [vpair(v_rest[m:m + 2]) for m in range(0, len(v_rest), 2)]
                    if i + 1 < NCH:
                        fill += qk_pieces(1, i + 1, j, 4)
                    ni, nj = (i, j + 1) if j + 1 < NB else (i + 1, 0)
                    if ni < NCH and (ni, nj) not in emitted_q:
                        fill += qk_pieces(0, ni, nj, 4)
                        emitted_q.add((ni, nj))
                    po0, po1 = attention_mms(i, j, filler=fill)
                    attention_norm(i, j, po0, po1)
        else:
            for j in range(1, NB):
                qk_block(0, 0, j)
            for i in range(1, NCH):
                for j in range(NB):
                    qk_block(1, i, j)
                    qk_block(0, i, j)
        if phases >= 3:
            start = (NB - 2) * NPB if phases >= 2 else 0
            for jj in range(start, NPC):
                phase3_rows(jj)
    elif drive_mode == "H":
        # nh0-first: emit only work whose inputs live in the first n-half
        # before the first attention block, so exp starts while the second
        # half of x is still being cast/transposed.
        NBH = max(1, NB // 2)      # n_q blocks per half
        NPH = NPC // 2             # V chunks per half
        for j in range(NBH):
            qk_block(1, 0, j)
        qk_block(0, 0, 0)
        for j in range(NPH):
            v_chunk(j)
        if phases >= 2:
            attention_pair_block(0, 0)
            for j in range(NBH, NB):
                qk_block(1, 0, j)
            for j in range(NPH, NPC):
                v_chunk(j)
            for i in range(NCH):
                for j in range(NB):
                    if not (i == 0 and j == 0):
                        qk_block(0, i, j)
                        attention_pair_block(i, j)
                    if i + 1 < NCH:
                        qk_block(1, i + 1, j)
        else:
            for j in range(NBH, NB):
                qk_block(1, 0, j)
            for j in range(NPH, NPC):
                v_chunk(j)
            for j in range(1, NB):
                qk_block(0, 0, j)
            for i in range(1, NCH):
                for j in range(NB):
                    qk_block(1, i, j)
                    qk_block(0, i, j)
        if phases >= 3:
            for jj in range(NPC):
                phase3_rows(jj)
    elif drive_mode == "G":
        # E + phase3 interleaved with the final attention blocks only
        for j in range(NB):
            qk_block(1, 0, j)
        qk_block(0, 0, 0)
        for j in range(NPC):
            v_chunk(j)
        if phases >= 2:
            for j in range(NB):
                if j > 0:
                    qk_block(0, 0, j)
                attention_pair_block(0, j)
                qk_block(1, 1, j)
            for j in range(NB):
                qk_block(0, 1, j)
            for j in range(NB):
                attention_pair_block(1, j)
                if phases >= 3:
                    for jj in range(j * NPB, (j + 1) * NPB):
                        phase3_rows(jj)
    else:  # C: K-first interleaved (previous)
        for i in range(NCH):
            for j in range(NB):
                qk_block(1, i, j)
            qk_block(0, i, 0)
            if i == 0:
                for j in range(NPC):
                    v_chunk(j)
            if phases < 2:
                for j in range(1, NB):
                    qk_block(0, i, j)
                continue
            for j in range(NB):
                if j > 0:
                    qk_block(0, i, j)
                attention_pair_block(i, j)
                if phases >= 3 and i == LH:
                    for jj in range(j * NPB, (j + 1) * NPB):
                        phase3_rows(jj)


def _split_perm(D):
    return np.concatenate([np.arange(0, D, 2), np.arange(1, D, 2)])


def _fp8_split(a, f8):
    """Split f32 array into two fp8(e4m3) terms: a ~= a1 + a2."""
    a1 = np.ascontiguousarray(a).astype(f8)
    a2 = np.ascontiguousarray(a - a1.astype(np.float32)).astype(f8)
    return a1, a2


def _prep_core_inputs(x, freqs_cis, w_qkv, w_proj, b, heads):
    perm = _split_perm(D)
    qrows, krows = [], []
    for h in heads:
        qrows.append(w_qkv[h * D:(h + 1) * D][perm])
        krows.append(w_qkv[C + h * D:C + (h + 1) * D][perm])
    vrows = [w_qkv[2 * C + h * D:2 * C + (h + 1) * D] for h in heads]
    wqk = np.concatenate(qrows + krows, axis=0)
    wv = np.concatenate(vrows, axis=0)
    hcols = np.concatenate([np.arange(h * D, (h + 1) * D) for h in heads])
    import ml_dtypes
    bf16 = ml_dtypes.bfloat16
    f8 = ml_dtypes.float8_e4m3
    x1, x2 = _fp8_split(x[b].T, f8)
    wqk1, wqk2 = _fp8_split(wqk.T * WS, f8)
    wv1, wv2 = _fp8_split(wv.T * WS, f8)
    return {
        "x1": x1, "x2": x2,
        "wqk1": wqk1, "wqk2": wqk2,
        "wv1": wv1, "wv2": wv2,
        "wpT": np.ascontiguousarray(w_proj[:, hcols].T / WS).astype(bf16),
        "cosT": np.ascontiguousarray(freqs_cis[:, :, 0].T).astype(bf16),
        "sinT": np.ascontiguousarray(
            np.concatenate([freqs_cis[:, :, 1].T, -freqs_cis[:, :, 1].T], axis=0)
        ).astype(bf16),
    }


_CACHE = {}


def _get_compiled():
    if "nc" not in _CACHE:
        nc = bacc.Bacc("TRN2", target_bir_lowering=False, debug=False)
        with tile.TileContext(nc) as tc:
            with ExitStack() as ctx:
                build_attn_kernel(nc, tc, ctx, N=N, C=C, HPC=HPC, D=D, NQ_BLK=512)
        nc.compile()
        _CACHE["nc"] = nc
    return _CACHE["nc"]


def make_in_maps(x, freqs_cis, w_qkv, w_proj):
    x = np.asarray(x, dtype=np.float32)
    freqs_cis = np.asarray(freqs_cis, dtype=np.float32)
    w_qkv = np.asarray(w_qkv, dtype=np.float32)
    w_proj = np.asarray(w_proj, dtype=np.float32)
    in_maps = []
    for c in range(N_CORES):
        b = c // CORES_PER_BATCH
        hg = c % CORES_PER_BATCH
        heads = list(range(hg * HPC, (hg + 1) * HPC))
        in_maps.append(_prep_core_inputs(x, freqs_cis, w_qkv, w_proj, b, heads))
    return in_maps


def gather_output(results, b_proj):
    out = np.zeros((B, N, C), dtype=np.float32)
    for c in range(N_CORES):
        out[c // CORES_PER_BATCH] += results[c]["y"].astype(np.float32)
    out += np.asarray(b_proj, dtype=np.float32)[None, None, :]
    return out


def kernel(x, freqs_cis, w_qkv, w_proj, b_proj):
    nc = _get_compiled()
    in_maps = make_in_maps(x, freqs_cis, w_qkv, w_proj)
    res = run_bass_kernel_spmd(nc, in_maps, core_ids=list(range(N_CORES)))
    return gather_output(res.results, b_proj)



# revision 125
# speedup vs baseline: 1.0001x; 1.0001x over previous
"""Trainium2 Bass kernel for nn_Attention_39015482916872.

Multi-head attention (B=2, N=2048, C=1024, H=16, D=64) with RoPE,
tensor-parallel over (batch, heads) across 8 NeuronCores: core c handles
batch c//4 and heads 4*(c%4)..4*(c%4)+3. Each core computes its heads'
QKV projection, RoPE, attention, and a partial output projection; the
host sums the 4 partials per batch (Megatron-style column-parallel
w_proj) and adds b_proj.

Device-side design notes:
 - All matmuls in bf16 (f32 accumulation in PSUM).
 - x is cast f32->bf16 by a DMA (SWDGE cast) into internal DRAM, then
   DMA-transposed (XBAR) into SBUF so C lands on partitions.
 - RoPE pairing (even,odd) is conjugated by a fixed permutation into
   [real32 | imag32] halves per head (folded into w_qkv rows host-side;
   cancels in q.k since Q and K share it) so the DVE ops stay
   32/64-partition aligned.
 - Scores are computed transposed (n_k on partitions); softmax uses no
   max-subtraction (scores ~ N(0,1), exp cannot overflow) and the
   denominator comes from a 65th all-ones column appended to V; the
   division is applied to the small (D x n_q) PV output.
"""

import sys
from contextlib import ExitStack, nullcontext

import numpy as np

if "/opt/trn_rl_repo" not in sys.path:
    sys.path.insert(0, "/opt/trn_rl_repo")
try:
    import concourse.bass as bass
except ImportError:
    sys.path.insert(0, "/root/.axon_site/_ro/trn_rl_repo")
    import concourse.bass as bass
import concourse.tile as tile
from concourse import bacc, mybir
from concourse.bass_utils import run_bass_kernel_spmd

F32 = mybir.dt.float32
BF16 = mybir.dt.bfloat16
F8 = mybir.dt.float8e4
AF = mybir.ActivationFunctionType
DR = mybir.MatmulPerfMode.DoubleRow

B, N, C, H, D = 2, 2048, 1024, 16, 64
N_CORES = 8
CORES_PER_BATCH = N_CORES // B          # 4
HPC = H // CORES_PER_BATCH              # 4 heads per core
WS = 32.0  # weight pre-scale: w_qkv ~ N(0, 1/32) -> ~N(0,1) avoids fp8 subnormals


def build_attn_kernel(nc, tc, ctx, N=2048, C=1024, HPC=4, D=64, NQ_BLK=512, scale=None, phases=3, drive_mode='E', SB=2):
    P = 128
    KC = C // P
    QK_CHUNKS = 2 * HPC * D // P
    VF = HPC * D
    NB = N // NQ_BLK
    NKC = N // P
    NPC = N // P
    if scale is None:
        # q and k each carry a WS factor from the pre-scaled weights
        scale = D ** -0.5 / (WS * WS)

    # QKV projection runs in fp8 (e4m3) with residual correction:
    # x = x1 + x2, w = w1 + w2 (both fp8 splits computed host-side), and the
    # psum accumulates x1w1 + x1w2 + x2w1 via DoubleRow matmuls (2 K-subtiles
    # per instruction at 0.5 cycles/row = 2x bf16 throughput). The dropped
    # x2w2 term is O(eps^2) ~ 0.1%.
    x1d = nc.dram_tensor("x1", [C, N], F8, kind="ExternalInput").ap()
    x2d = nc.dram_tensor("x2", [C, N], F8, kind="ExternalInput").ap()
    wqk1d = nc.dram_tensor("wqk1", [C, 2 * HPC * D], F8, kind="ExternalInput").ap()
    wqk2d = nc.dram_tensor("wqk2", [C, 2 * HPC * D], F8, kind="ExternalInput").ap()
    wv1d = nc.dram_tensor("wv1", [C, VF], F8, kind="ExternalInput").ap()
    wv2d = nc.dram_tensor("wv2", [C, VF], F8, kind="ExternalInput").ap()
    wpT = nc.dram_tensor("wpT", [VF, C], BF16, kind="ExternalInput").ap()
    cosT = nc.dram_tensor("cosT", [D // 2, N], BF16, kind="ExternalInput").ap()
    sinT = nc.dram_tensor("sinT", [D, N], BF16, kind="ExternalInput").ap()
    y = nc.dram_tensor("y", [N, C], BF16, kind="ExternalOutput").ap()

    persist = ctx.enter_context(tc.tile_pool(name="persist", bufs=1))
    psum_mm = ctx.enter_context(tc.tile_pool(name="psum_mm", bufs=2, space="PSUM"))
    # psum_s / psum_o are created lazily (after the boot pool closes) so the
    # opening QKV phase can use their banks for deeper psum pipelining
    psum_s = psum_o = None

    def make_attn_psum_pools():
        nonlocal psum_s, psum_o
        if psum_s is None:
            psum_s = ctx.enter_context(tc.tile_pool(name="psum_s", bufs=2, space="PSUM"))
            psum_o = ctx.enter_context(tc.tile_pool(name="psum_o", bufs=1, space="PSUM"))
    rope_tmp = ctx.enter_context(tc.tile_pool(name="rope_tmp", bufs=6))
    exp_pool = ctx.enter_context(tc.tile_pool(name="exp_pool", bufs=34))
    norm_pool = ctx.enter_context(tc.tile_pool(name="norm_pool", bufs=6))
    y_pool = ctx.enter_context(tc.tile_pool(name="y_pool", bufs=8))

    NH = max(1, N // 1024)   # n-halves of 1024
    HW_ = N // NH            # half width
    KH = KC // 2             # DoubleRow k-subtile pairs
    x1sb = [persist.tile([P, KC, HW_], F8, name=f"x1_{h}", tag=f"x1_{h}") for h in range(NH)]
    x2sb = [persist.tile([P, KC, HW_], F8, name=f"x2_{h}", tag=f"x2_{h}") for h in range(NH)]

    def x_slice(xsb, k2, n0, w):
        h = n0 // HW_
        assert (n0 + w - 1) // HW_ == h
        return xsb[h][:, 2 * k2:2 * k2 + 2, n0 - h * HW_:n0 - h * HW_ + w]
    wqk1_sb = persist.tile([P, KC, 2 * HPC * D], F8, tag="wqk1")
    wqk2_sb = persist.tile([P, KC, 2 * HPC * D], F8, tag="wqk2")
    wv1_sb = persist.tile([P, KC, VF], F8, tag="wv1")
    wv2_sb = persist.tile([P, KC, VF], F8, tag="wv2")
    wpT_sb = persist.tile([P, VF // P, C], BF16, tag="wp")
    cos_sb = persist.tile([P, N], BF16, tag="cos")
    sin_sb = persist.tile([P, N], BF16, tag="sin")
    qt = [[persist.tile([P, NQ_BLK], BF16, name=f"qt{i}_{j}", tag=f"qt{i}_{j}") for j in range(NB)] for i in range(QK_CHUNKS // 2)]
    kt = [[persist.tile([P, NQ_BLK], BF16, name=f"kt{i}_{j}", tag=f"kt{i}_{j}") for j in range(NB)] for i in range(QK_CHUNKS // 2)]
    # V augmented with 64 ones-columns: MM2 then yields [v-out | denominator
    # replicated across 64 partitions], so softmax normalize needs no
    # partition-broadcast (matmul cost is per moving row -- the extra PE
    # columns are free)
    vaug = [persist.tile([P, HPC, 2 * D], BF16, name=f"va{j}", tag=f"va{j}") for j in range(NPC)]
    # anorm split per (feature-chunk, nq-block) so phase3 readers only wait on
    # the normalize writes of their own nq block (Tile deps are tile-granular)
    anorm = [[persist.tile([P, NQ_BLK], BF16, name=f"an{i}_{j}", tag=f"an{i}_{j}")
              for j in range(NB)] for i in range(VF // P)]

    def anorm_slice(i, n0, w):
        j = n0 // NQ_BLK
        assert (n0 + w - 1) // NQ_BLK == j
        return anorm[i][j][:, n0 - j * NQ_BLK:n0 - j * NQ_BLK + w]

    # preload the exp activation table during the DMA prefix so the first
    # real softmax exp doesn't pay the ~2.7us ACT_TABLE_LOAD
    warm = persist.tile([1, 8], F32, tag="actwarm")
    nc.vector.memset(warm[:], 0.0)
    nc.scalar.activation(warm[:], warm[:], AF.Exp, scale=1.0)

    # DMA order tuned so the first qk_block's term-1 dependencies (wqk1 +
    # x1[half0]) land first, then term-2/3 operands, then cos/sin (needed by
    # the first rope), then half-1 x with wv, then wp (only needed in phase 3).
    XB = 2  # x chunks per DMA (batches HWDGE/SP issue overhead)

    def _xdma(xd, xsb, h, k, w=None):
        n0 = h * HW_
        w = w or XB
        nc.sync.dma_start(
            xsb[h][:, k:k + w, :],
            xd[k * P:(k + w) * P, n0:n0 + HW_].rearrange("(kc p) n -> p kc n", p=P),
        )

    HK = KC // 2
    nc.sync.dma_start(wqk1_sb[:, 0:HK, :], wqk1d[0:HK * P, :].rearrange("(kc p) f -> p kc f", p=P))
    for k in range(0, HK, XB):
        _xdma(x1d, x1sb, 0, k)
    nc.sync.dma_start(wqk1_sb[:, HK:KC, :], wqk1d[HK * P:, :].rearrange("(kc p) f -> p kc f", p=P))
    for k in range(HK, KC, XB):
        _xdma(x1d, x1sb, 0, k)
    nc.sync.dma_start(wqk2_sb[:], wqk2d.rearrange("(kc p) f -> p kc f", p=P))
    for k in range(0, KC, XB):
        _xdma(x2d, x2sb, 0, k)
    for g in range(P // (D // 2)):
        nc.sync.dma_start(cos_sb[g * (D // 2):(g + 1) * (D // 2), :], cosT[:, :])
    for g in range(P // D):
        nc.sync.dma_start(sin_sb[g * D:(g + 1) * D, :], sinT[:, :])
    nc.sync.dma_start(wv1_sb[:], wv1d.rearrange("(kc p) f -> p kc f", p=P))
    nc.sync.dma_start(wv2_sb[:], wv2d.rearrange("(kc p) f -> p kc f", p=P))
    if NH > 1:
        for k in range(0, KC, 4):
            _xdma(x1d, x1sb, 1, k, w=4)
            _xdma(x2d, x2sb, 1, k, w=4)
    nc.sync.dma_start(wpT_sb[:], wpT.rearrange("(vc p) f -> p vc f", p=P))
    HALF0_FIRST = True

    QKV_TERMS = 3  # (x1,w1), (x1,w2), (x2,w1)

    def v_chunk(j, pool=None, evict_dve=False):
        pv = (pool or psum_mm).tile([P, VF], F32, name="pmm", tag="pmm", padded_shape=[P, 512])
        for t, (xsb, wsb) in enumerate(((x1sb, wv1_sb), (x1sb, wv2_sb), (x2sb, wv1_sb))):
            for k2 in range(KH):
                nc.tensor.matmul(
                    pv[:],
                    lhsT=x_slice(xsb, k2, j * P, P),
                    rhs=wsb[:, 2 * k2:2 * k2 + 2, :],
                    start=(t == 0 and k2 == 0),
                    stop=(t == QKV_TERMS - 1 and k2 == KH - 1),
                    perf_mode=DR,
                )
        nc.vector.memset(vaug[j][:, :, D:2 * D], 1.0)
        if evict_dve:
            nc.vector.tensor_copy(vaug[j][:, :, 0:D], pv[:].rearrange("p (h d) -> p h d", d=D))
        else:
            nc.any.tensor_copy(vaug[j][:, :, 0:D], pv[:].rearrange("p (h d) -> p h d", d=D))

    if phases < 0.4:
        return
    if phases < 0.8:
        for j in range(NPC):
            v_chunk(j)
        return
    # Phase 1b: Q,K feat-major + RoPE (chunk feat layout: [h0r h0i h1r h1i])
    def rope_chunk(psum_c, dst, j, raw_on_dve=False, eng=None):
        eng = eng or nc.vector
        nb = j * NQ_BLK
        cs = cos_sb[:, nb:nb + NQ_BLK]
        sn = sin_sb[:, nb:nb + NQ_BLK]
        raw = rope_tmp.tile([P, NQ_BLK], BF16, tag="raw")
        if raw_on_dve:
            # inside the Act-bound attention region every Act cycle counts
            nc.vector.tensor_copy(raw[:], psum_c[:])
        else:
            nc.any.tensor_copy(raw[:], psum_c[:])
        tA = rope_tmp.tile([P, NQ_BLK], BF16, tag="tA")
        tB = rope_tmp.tile([P, NQ_BLK], BF16, tag="tB")
        eng.tensor_mul(tA[:], raw[:], cs)
        # swapped sin product: out rows swap r<->i; the +/- sign is folded into
        # the sin table (rows [g*64:g*64+32] = +sin, [g*64+32:g*64+64] = -sin),
        # so DVE 2-input base partitions always match (walrus NCC_IBIR297).
        for g in range(2):
            b0 = 64 * g
            eng.tensor_mul(tB[b0:b0 + 32, :], raw[b0 + 32:b0 + 64, :], sn[b0 + 32:b0 + 64, :])
            eng.tensor_mul(tB[b0 + 32:b0 + 64, :], raw[b0:b0 + 32, :], sn[b0:b0 + 32, :])
        eng.tensor_add(dst[:], tA[:], tB[:])

    def qk_pieces(qk, i, j, n_pieces):
        """Split a qk_block's 12 DoubleRow matmuls (+rope) into n_pieces
        emission thunks, for interleaving into an attention block's kk loop
        (PE filler while the Act engine works through the exp backlog)."""
        dst_t = qt[i][j] if qk == 0 else kt[i][j]
        fbase = (qk * (QK_CHUNKS // 2) + i) * P
        pairs = [(t, k2)
                 for t in range(QKV_TERMS) for k2 in range(KH)]
        terms = ((x1sb, wqk1_sb), (x1sb, wqk2_sb), (x2sb, wqk1_sb))
        cell = {}
        chunk = -(-len(pairs) // n_pieces)

        def make(p):
            def emit():
                if "pq" not in cell:
                    cell["pq"] = psum_mm.tile([P, NQ_BLK], F32, name="pmm", tag="pmm", padded_shape=[P, 512])
                pqk = cell["pq"]
                for idx in range(p * chunk, min((p + 1) * chunk, len(pairs))):
                    t, k2 = pairs[idx]
                    xsb, wsb = terms[t]
                    nc.tensor.matmul(
                        pqk[:],
                        lhsT=wsb[:, 2 * k2:2 * k2 + 2, fbase:fbase + P],
                        rhs=x_slice(xsb, k2, j * NQ_BLK, NQ_BLK),
                        start=(idx == 0),
                        stop=(idx == len(pairs) - 1),
                        perf_mode=DR,
                    )
                if (p + 1) * chunk >= len(pairs):
                    rope_chunk(pqk, dst_t[:], j, raw_on_dve=True)
            return emit
        return [make(p) for p in range(n_pieces)]

    def qk_block(qk, i, j, pool=None, rope_eng=None):
        dst_t = qt[i][j] if qk == 0 else kt[i][j]
        fbase = (qk * (QK_CHUNKS // 2) + i) * P
        pqk = (pool or psum_mm).tile([P, NQ_BLK], F32, name="pmm", tag="pmm", padded_shape=[P, 512])
        for t, (xsb, wsb) in enumerate(((x1sb, wqk1_sb), (x1sb, wqk2_sb), (x2sb, wqk1_sb))):
            for k2 in range(KH):
                nc.tensor.matmul(
                    pqk[:],
                    lhsT=wsb[:, 2 * k2:2 * k2 + 2, fbase:fbase + P],
                    rhs=x_slice(xsb, k2, j * NQ_BLK, NQ_BLK),
                    start=(t == 0 and k2 == 0),
                    stop=(t == QKV_TERMS - 1 and k2 == KH - 1),
                    perf_mode=DR,
                )
        rope_chunk(pqk, dst_t[:], j, eng=rope_eng)

    # Phase 2: attention per head, scores transposed (n_k on partitions).
    # psum_s batches SB nk-chunks so each exp covers SB*NQ_BLK elements
    # (amortizes the ~352-cycle ACT per-instruction overhead).
    def attention_mms(i, j, filler=(), filler_at=None, hi=False):
        # Both heads of chunk i at n_q block j. The two MM1s at each kk use
        # disjoint PE row strips (rows 0-63 vs 64-127), issued back-to-back so
        # the hardware runs them concurrently; one exp covers both heads.
        # `filler` thunks (next qk blocks' matmul pieces) are emitted spread
        # through the kk loop: the attention stretch is locally Act-bound, so
        # they fill PE slots while MM1 waits for the exp to free psum_s.
        h0, h1 = 2 * i, 2 * i + 1
        po0 = psum_o.tile([2 * D, NQ_BLK], F32, name="po0", tag="po0")
        po1 = psum_o.tile([2 * D, NQ_BLK], F32, name="po1", tag="po1")
        nf = len(filler)
        for kk in range(NKC):
            if filler_at:
                for th in filler_at.get(kk, ()):
                    th()
            with tc.high_priority() if hi else nullcontext():
                ps = psum_s.tile([P, 2, NQ_BLK], F32, tag="ps")
                kb, kc0 = divmod(kk * P, NQ_BLK)
                for g, h in ((0, h0), (1, h1)):
                    hb = 64 * g
                    nc.tensor.matmul(
                        ps[:, g, :],
                        lhsT=kt[i][kb][hb:hb + 64, kc0:kc0 + P],
                        rhs=qt[i][j][hb:hb + 64, :],
                        start=True,
                        stop=True,
                    )
                es = exp_pool.tile([P, 2, NQ_BLK], BF16, tag="es")
                nc.scalar.activation(es[:], ps[:], AF.Exp, scale=float(scale))
                for g, h, po in ((0, h0, po0), (1, h1, po1)):
                    nc.tensor.matmul(
                        po[:],
                        lhsT=vaug[kk][:, h, :],
                        rhs=es[:, g, :],
                        start=(kk == 0),
                        stop=(kk == NKC - 1),
                    )
            if nf:
                for p in range(nf):
                    if ((p + 1) * NKC) // nf - 1 == kk:
                        filler[p]()
        return po0, po1

    def attention_norm(i, j, po0, po1):
        # recip on DVE, multiply on gpsimd: anorm's writer engine is Pool, so
        # phase3's coarse engine-counter waits don't drag in the DVE rope queue
        h0, h1 = 2 * i, 2 * i + 1
        for h, po in ((h0, po0), (h1, po1)):
            bcast = norm_pool.tile([D, NQ_BLK], F32, tag="bcast")
            nc.vector.reciprocal(bcast[:], po[D:2 * D, :])
            dst = anorm[(h * D) // P][j]
            db = (h * D) % P
            nc.vector.tensor_mul(dst[db:db + D, :], po[0:D, :], bcast[:])

    def attention_pair_block(i, j):
        po0, po1 = attention_mms(i, j)
        attention_norm(i, j, po0, po1)


    # Phase 3: partial output projection (natural layout, n on partitions)
    OB = min(512, C)
    NOB = C // OB
    def phase3_rows(j, evict_dve=False):
        for ob in range(NOB):
            py = psum_mm.tile([P, OB], F32, name="pmm", tag="pmm", padded_shape=[P, 512])
            for i in range(VF // P):
                nc.tensor.matmul(
                    py[:],
                    lhsT=anorm_slice(i, j * P, P),
                    rhs=wpT_sb[:, i, ob * OB:(ob + 1) * OB],
                    start=(i == 0),
                    stop=(i == VF // P - 1),
                )
            yt = y_pool.tile([P, OB], BF16, tag="yt")
            if evict_dve:
                nc.vector.tensor_copy(yt[:], py[:])
            else:
                nc.any.tensor_copy(yt[:], py[:])
            nc.sync.dma_start(y[j * P:(j + 1) * P, ob * OB:(ob + 1) * OB], yt[:])

    NPB = NQ_BLK // P       # 128-row chunks per nq block
    NCH = QK_CHUNKS // 2
    LH = NCH - 1
    if drive_mode != "E":
        make_attn_psum_pools()
    if drive_mode in ("A", "D"):
        # V, then per chunk: K, Q, attention; phase3 interleaved (A) or last (D)
        for j in range(NPC):
            v_chunk(j)
        for i in range(NCH):
            for j in range(NB):
                qk_block(1, i, j)
            for j in range(NB):
                qk_block(0, i, j)
            if phases < 2:
                continue
            for j in range(NB):
                attention_pair_block(i, j)
                if phases >= 3 and i == LH and drive_mode == "A":
                    for jj in range(j * NPB, (j + 1) * NPB):
                        phase3_rows(jj)
        if phases >= 3 and drive_mode == "D":
            for jj in range(NPC):
                phase3_rows(jj)
    elif drive_mode == "B":
        # V, all QK chunks, then all attention
        for j in range(NPC):
            v_chunk(j)
        for i in range(NCH):
            for j in range(NB):
                qk_block(1, i, j)
            for j in range(NB):
                qk_block(0, i, j)
        if phases >= 2:
            for i in range(NCH):
                for j in range(NB):
                    attention_pair_block(i, j)
                    if phases >= 3 and i == LH:
                        for jj in range(j * NPB, (j + 1) * NPB):
                            phase3_rows(jj)
    elif drive_mode == "E":
        # earliest-exp: half-0 QKV in a deep scoped psum pool, then half-1
        # QK; second-half V chunks ride as fillers inside attention(0,0)
        # (only needed from kk=8). Remaining qk blocks are interleaved into
        # the attention kk loops as PE filler (the attention stretch is
        # Act-bound, so filler matmuls are free).
        pre = set()
        NBH = NB // 2
        # Only the first 4 (DMA-paced) qk blocks use the scoped boot pool:
        # psum_s/psum_o allocate in the released boot zone, so the boot pool
        # must close EARLY or its release dependency gates the first exp.
        # Later boot work (V chunks, half-1 K blocks) runs on psum_mm and
        # overlaps the attention stream.
        with tc.tile_pool(name="psum_boot", bufs=6, space="PSUM") as boot:
            qk_block(1, 0, 0, pool=boot)
            qk_block(0, 0, 0, pool=boot)
            # chains for block 1 (needed only from kk4) run on the idle
            # gpsimd engine, in parallel with the MM1-critical DVE chains
            qk_block(1, 0, 1, pool=boot, rope_eng=nc.gpsimd)
            qk_block(0, 0, 1, pool=boot, rope_eng=nc.gpsimd)
            pre.update(range(NBH))
        make_attn_psum_pools()
        for j in range(NPC // 2):
            v_chunk(j)
        for j in range(NBH, NB):
            qk_block(1, 0, j)
        v_rest = list(range(NPC // 2, NPC))
        if phases >= 2:
            emitted_q = {(0, j) for j in pre}
            for i in range(NCH):
                for j in range(NB):
                    if (i, j) not in emitted_q:
                        qk_block(0, i, j)
                    fill = []
                    if phases >= 3 and i == NCH - 1 and j >= 2:
                        def p3pair(jjs):
                            def emit():
                                for jj in jjs:
                                    phase3_rows(jj, evict_dve=True)
                            return emit
                        base = (j - 2) * NPB
                        fill += [p3pair([base, base + 1]), p3pair([base + 2, base + 3])]
                    if i == 0 and j == 0 and v_rest:
                        def vpair(js):
                            def emit():
                                for jj in js:
                                    v_chunk(jj)
                            return emit
                        fill += [vpair(v_rest[m:m + 2]) for m in range(0, len(v_rest), 2)]
                    if i + 1 < NCH:
                        fill += qk_pieces(1, i + 1, j, 4)
                    ni, nj = (i, j + 1) if j + 1 < NB else (i + 1, 0)
                    if ni < NCH and (ni, nj) not in emitted_q:
                        fill += qk_pieces(0, ni, nj, 4)
                        emitted_q.add((ni, nj))
                    po0, po1 = attention_mms(i, j, filler=fill)
                    attention_norm(i, j, po0, po1)
        else:
            for j in range(1, NB):
                qk_block(0, 0, j)
            for i in range(1, NCH):
                for j in range(NB):
                    qk_block(1, i, j)
                    qk_block(0, i, j)
        if phases >= 3:
            start = (NB - 2) * NPB if phases >= 2 else 0
            for jj in range(start, NPC):
                phase3_rows(jj)
    elif drive_mode == "H":
        # nh0-first: emit only work whose inputs live in the first n-half
        # before the first attention block, so exp starts while the second
        # half of x is still being cast/transposed.
        NBH = max(1, NB // 2)      # n_q blocks per half
        NPH = NPC // 2             # V chunks per half
        for j in range(NBH):
            qk_block(1, 0, j)
        qk_block(0, 0, 0)
        for j in range(NPH):
            v_chunk(j)
        if phases >= 2:
            attention_pair_block(0, 0)
            for j in range(NBH, NB):
                qk_block(1, 0, j)
            for j in range(NPH, NPC):
                v_chunk(j)
            for i in range(NCH):
                for j in range(NB):
                    if not (i == 0 and j == 0):
                        qk_block(0, i, j)
                        attention_pair_block(i, j)
                    if i + 1 < NCH:
                        qk_block(1, i + 1, j)
        else:
            for j in range(NBH, NB):
                qk_block(1, 0, j)
            for j in range(NPH, NPC):
                v_chunk(j)
            for j in range(1, NB):
                qk_block(0, 0, j)
            for i in range(1, NCH):
                for j in range(NB):
                    qk_block(1, i, j)
                    qk_block(0, i, j)
        if phases >= 3:
            for jj in range(NPC):
                phase3_rows(jj)
    elif drive_mode == "G":
        # E + phase3 interleaved with the final attention blocks only
        for j in range(NB):
            qk_block(1, 0, j)
        qk_block(0, 0, 0)
        for j in range(NPC):
            v_chunk(j)
        if phases >= 2:
            for j in range(NB):
                if j > 0:
                    qk_block(0, 0, j)
                attention_pair_block(0, j)
                qk_block(1, 1, j)
            for j in range(NB):
                qk_block(0, 1, j)
            for j in range(NB):
                attention_pair_block(1, j)
                if phases >= 3:
                    for jj in range(j * NPB, (j + 1) * NPB):
                        phase3_rows(jj)
    else:  # C: K-first interleaved (previous)
        for i in range(NCH):
            for j in range(NB):
                qk_block(1, i, j)
            qk_block(0, i, 0)
            if i == 0:
                for j in range(NPC):
                    v_chunk(j)
            if phases < 2:
                for j in range(1, NB):
                    qk_block(0, i, j)
                continue
            for j in range(NB):
                if j > 0:
                    qk_block(0, i, j)
                attention_pair_block(i, j)
                if phases >= 3 and i == LH:
                    for jj in range(j * NPB, (j + 1) * NPB):
                        phase3_rows(jj)


def _split_perm(D):
    return np.concatenate([np.arange(0, D, 2), np.arange(1, D, 2)])


def _fp8_split(a, f8):
    """Split f32 array into two fp8(e4m3) terms: a ~= a1 + a2."""
    a1 = np.ascontiguousarray(a).astype(f8)
    a2 = np.ascontiguousarray(a - a1.astype(np.float32)).astype(f8)
    return a1, a2


def _prep_core_inputs(x, freqs_cis, w_qkv, w_proj, b, heads):
    perm = _split_perm(D)
    qrows, krows = [], []
    for h in heads:
        qrows.append(w_qkv[h * D:(h + 1) * D][perm])
        krows.append(w_qkv[C + h * D:C + (h + 1) * D][perm])
    vrows = [w_qkv[2 * C + h * D:2 * C + (h + 1) * D] for h in heads]
    wqk = np.concatenate(qrows + krows, axis=0)
    wv = np.concatenate(vrows, axis=0)
    hcols = np.concatenate([np.arange(h * D, (h + 1) * D) for h in heads])
    import ml_dtypes
    bf16 = ml_dtypes.bfloat16
    f8 = ml_dtypes.float8_e4m3
    x1, x2 = _fp8_split(x[b].T, f8)
    wqk1, wqk2 = _fp8_split(wqk.T * WS, f8)
    wv1, wv2 = _fp8_split(wv.T * WS, f8)
    return {
        "x1": x1, "x2": x2,
        "wqk1": wqk1, "wqk2": wqk2,
        "wv1": wv1, "wv2": wv2,
        "wpT": np.ascontiguousarray(w_proj[:, hcols].T / WS).astype(bf16),
        "cosT": np.ascontiguousarray(freqs_cis[:, :, 0].T).astype(bf16),
        "sinT": np.ascontiguousarray(
            np.concatenate([freqs_cis[:, :, 1].T, -freqs_cis[:, :, 1].T], axis=0)
        ).astype(bf16),
    }


_CACHE = {}


def _get_compiled():
    if "nc" not in _CACHE:
        nc = bacc.Bacc("TRN2", target_bir_lowering=False, debug=False)
        with tile.TileContext(nc) as tc:
            with ExitStack() as ctx:
                build_attn_kernel(nc, tc, ctx, N=N, C=C, HPC=HPC, D=D, NQ_BLK=512)
        nc.compile()
        _CACHE["nc"] = nc
    return _CACHE["nc"]


def make_in_maps(x, freqs_cis, w_qkv, w_proj):
    x = np.asarray(x, dtype=np.float32)
    freqs_cis = np.asarray(freqs_cis, dtype=np.float32)
    w_qkv = np.asarray(w_qkv, dtype=np.float32)
    w_proj = np.asarray(w_proj, dtype=np.float32)
    in_maps = []
    for c in range(N_CORES):
        b = c // CORES_PER_BATCH
        hg = c % CORES_PER_BATCH
        heads = list(range(hg * HPC, (hg + 1) * HPC))
        in_maps.append(_prep_core_inputs(x, freqs_cis, w_qkv, w_proj, b, heads))
    return in_maps


def gather_output(results, b_proj):
    out = np.zeros((B, N, C), dtype=np.float32)
    for c in range(N_CORES):
        out[c // CORES_PER_BATCH] += results[c]["y"].astype(np.float32)
    out += np.asarray(b_proj, dtype=np.float32)[None, None, :]
    return out


def kernel(x, freqs_cis, w_qkv, w_proj, b_proj):
    nc = _get_compiled()
    in_maps = make_in_maps(x, freqs_cis, w_qkv, w_proj)
    res = run_bass_kernel_spmd(nc, in_maps, core_ids=list(range(N_CORES)))
    return gather_output(res.results, b_proj)

